# revision 1
# baseline (speedup 1.0000x reference)
"""LoopyBP kernel for 8 Trainium2 NeuronCores.

Strategy:
  - Edges are globally sorted by dst and packed into 8*128 partition
    stretches (node-run aligned) so the per-node segment sums become
    per-partition segmented scans (DVE tensor_tensor_scan), fully local.
  - Per iteration one SPMD bass launch computes, per edge slot s:
        Z[s]   = logQ[dst_s] - logm[s]      (fwd scan + reverse broadcast scan)
        W[s]   = normalize(max(exp(Z[s]),EPS) @ psi)   (psi = (a-b)I + bJ fast path)
    W[s] is the NEW message for edge rev(e_s) (rev is an involution: the
    reverse-edge message update only needs local, dst-sorted data).
  - Host applies the static slot permutation M_next = W[revslot] between
    launches (rev/src/dst are constant across iterations).
  - Final belief pass: one more scan launch + tiny host reduction.
Fallback: if rev is not an involution or psi is not (a-b)I+bJ, compute with
numpy exactly like the reference (correct, slow - not expected to trigger).
"""

import numpy as np

EPS = 1e-12
N_CORES = 8
P = 128
K = 7
EPP = 3280          # slots per partition stretch
CH = 164            # chunk width (EPP must be divisible)
NCH = EPP // CH
NSTRETCH = N_CORES * P

_compiled = {}


# --------------------------------------------------------------------------
# host-side layout
# --------------------------------------------------------------------------
def _build_layout(prior, src, dst, rev):
    n, k = prior.shape
    E = src.shape[0]
    order = np.argsort(dst, kind="stable")
    dsorted = dst[order]
    # node runs in sorted order
    uniq, run_start = np.unique(dsorted, return_index=True)
    run_len = np.diff(np.append(run_start, E))
    nruns = len(uniq)

    # greedy pack runs into stretches of EPP (node-aligned)
    stretch_of_run = np.empty(nruns, np.int64)
    pos_of_run = np.empty(nruns, np.int64)
    cur, fill = 0, 0
    for r in range(nruns):
        L = run_len[r]
        if fill + L > EPP:
            cur += 1
            fill = 0
            if cur >= NSTRETCH:
                raise RuntimeError("EPP too small for packing")
        stretch_of_run[r] = cur
        pos_of_run[r] = fill
        fill += L
    S_total = NSTRETCH * EPP

    # slot of each sorted-edge
    run_of_sorted = np.repeat(np.arange(nruns), run_len)
    off_in_run = np.arange(E) - run_start[run_of_sorted]
    slot_sorted = stretch_of_run[run_of_sorted] * EPP + pos_of_run[run_of_sorted] + off_in_run
    slot_of_edge = np.empty(E, np.int64)
    slot_of_edge[order] = slot_sorted

    real = np.zeros(S_total, bool)
    real[slot_sorted] = True

    # masks
    m0 = np.ones(S_total, np.float32)          # fwd scan carry mask: 0 at run starts
    em = np.zeros(S_total, np.float32)         # 1 at run ends
    startslot = stretch_of_run * EPP + pos_of_run
    endslot = startslot + run_len - 1
    m0[startslot] = 0.0
    m0[~real] = 0.0
    em[endslot] = 1.0
    ne = 1.0 - em                              # rev scan carry mask

    lp = np.zeros((S_total, K), np.float32)
    logprior = np.log(np.maximum(prior, 1e-30)).astype(np.float32)
    lp[slot_sorted] = logprior[dsorted]
    lp *= em[:, None]

    # between-launch permutation: M_next[s] = W[slot_of(rev(edge(s)))]
    revslot = np.arange(S_total, dtype=np.int64)
    revslot[slot_of_edge] = slot_of_edge[rev]

    # final extraction: logP[d] = S_final[endslot(run of d)]
    runend_of_node = np.full(n, -1, np.int64)
    runend_of_node[uniq] = endslot
    return dict(slot_of_edge=slot_of_edge, m0=m0, em=em, ne=ne, lp=lp,
                revslot=revslot, runend_of_node=runend_of_node, S_total=S_total)


# --------------------------------------------------------------------------
# device programs
# --------------------------------------------------------------------------
def _get_programs(alpha, beta):
    key = (round(float(alpha), 9), round(float(beta), 9))
    if key in _compiled:
        return _compiled[key]
    import concourse.bacc as bacc
    import concourse.mybir as mybir
    from concourse.tile import TileContext

    F32 = mybir.dt.float32
    Ln = mybir.ActivationFunctionType.Ln
    Exp = mybir.ActivationFunctionType.Exp
    Copy = mybir.ActivationFunctionType.Copy
    ADD = mybir.AluOpType.add
    MULT = mybir.AluOpType.mult
    SUB = mybir.AluOpType.subtract
    MIN = mybir.AluOpType.min

    gamma = (alpha - beta) / (alpha + 6.0 * beta)
    delta = beta / (alpha + 6.0 * beta)

    # ---------------- program A: one BP iteration -------------------------
    ncA = bacc.Bacc(None, num_devices=N_CORES)
    t_min = ncA.dram_tensor("min", [P, EPP * K], F32, kind="ExternalInput")
    t_lp = ncA.dram_tensor("lp", [P, EPP * K], F32, kind="ExternalInput")
    t_m0 = ncA.dram_tensor("m0", [P, EPP], F32, kind="ExternalInput")
    t_ne = ncA.dram_tensor("ne", [P, EPP], F32, kind="ExternalInput")
    t_em = ncA.dram_tensor("em", [P, EPP], F32, kind="ExternalInput")
    t_w = ncA.dram_tensor("w", [P, EPP * K], F32, kind="ExternalOutput")

    for _cv in (27.631021115928547, -27.631021115928547):
        _ct = ncA.alloc_sbuf_tensor(f"constf32_{_cv}".replace(".", "_").replace("-", "m"), [128, 1], F32)
        ncA.gpsimd.memset(_ct.ap(), _cv)
        ncA.const_aps.aps[(F32, _cv)] = _ct.ap()
    ncA.all_engine_barrier()

    with TileContext(ncA) as tc:
        with tc.tile_pool(name="big", bufs=1) as big, \
             tc.tile_pool(name="chp", bufs=4) as chp, \
             tc.tile_pool(name="chq", bufs=4) as chq:
            S = big.tile([P, EPP * K], F32, tag="S")
            M0 = big.tile([P, EPP], F32, tag="M0")
            NE = big.tile([P, EPP], F32, tag="NE")
            EM = big.tile([P, EPP], F32, tag="EM")
            ncA.sync.dma_start(M0[:], t_m0[:])
            ncA.sync.dma_start(NE[:], t_ne[:])
            ncA.sync.dma_start(EM[:], t_em[:])
            S3 = S[:].rearrange("p (e k) -> p e k", k=K)

            # phase 1: L = ln(M), S = segmented forward scan of L
            for c in range(NCH):
                a, b = c * CH, (c + 1) * CH
                mt = chp.tile([P, CH * K], F32, tag="mt")
                ncA.sync.dma_start(mt[:], t_min[:, a * K:b * K])
                ncA.scalar.activation(mt[:], mt[:], Ln)
                lt = chp.tile([P, CH * K], F32, tag="aux")
                ncA.sync.dma_start(lt[:], t_lp[:, a * K:b * K])
                ncA.vector.tensor_tensor(mt[:], mt[:], lt[:], ADD)
                mt3 = mt[:].rearrange("p (e k) -> p e k", k=K)
                for kk in range(K):
                    init = 0.0 if c == 0 else S3[:, a - 1:a, kk]
                    ncA.vector.tensor_tensor_scan(
                        S3[:, a:b, kk], M0[:, a:b], mt3[:, :, kk], init, MULT, ADD)

            # phase 3 (reverse chunk order): B = reverse broadcast scan of A,
            # then Z = B - L, b = max(exp(Z),EPS), W = normalize(psi fast path)
            prevB = None
            for c in range(NCH - 1, -1, -1):
                a, b = c * CH, (c + 1) * CH
                Bt = chq.tile([P, CH * K], F32, tag="Bt")
                Bt3 = Bt[:].rearrange("p (e k) -> p e k", k=K)
                for kk in range(K):
                    init = 0.0 if prevB is None else prevB[:, 0:1, kk]
                    ncA.vector.tensor_tensor_scan(
                        Bt3[:, ::-1, kk], NE[:, a:b][:, ::-1],
                        S3[:, a:b, kk][:, ::-1], init, MULT, MIN)
                prevB = Bt3
                mt = chp.tile([P, CH * K], F32, tag="aux")
                ncA.sync.dma_start(mt[:], t_min[:, a * K:b * K])
                ncA.scalar.activation(mt[:], mt[:], Ln)
                ncA.vector.tensor_tensor(mt[:], Bt[:], mt[:], SUB)   # Z = B - L
                Relu = mybir.ActivationFunctionType.Relu
                ncA.scalar.activation(mt[:], mt[:], Relu, bias=27.631021115928547)
                ncA.scalar.activation(mt[:], mt[:], Exp, bias=-27.631021115928547)  # b=exp(max(Z,lnEPS))
                mt3 = mt[:].rearrange("p (e k) -> p e k", k=K)
                s2 = chq.tile([P, CH], F32, tag="s2")
                ncA.vector.tensor_reduce(s2[:], mt3[:, :, :], mybir.AxisListType.X, ADD)
                r2 = chq.tile([P, CH], F32, tag="r2")
                ncA.vector.reciprocal(r2[:], s2[:])
                rb = r2[:].rearrange("p (e o) -> p e o", o=1).broadcast_to([P, CH, K])
                ncA.vector.tensor_tensor(mt3[:, :, :], mt3[:, :, :], rb, MULT)  # b/S2
                ncA.scalar.activation(mt[:], mt[:], Copy, bias=delta, scale=gamma)
                ncA.sync.dma_start(t_w[:, a * K:b * K], mt[:])
    ncA.compile()

    # ---------------- program B: final forward scan -----------------------
    ncB = bacc.Bacc(None, num_devices=N_CORES)
    b_min = ncB.dram_tensor("min", [P, EPP * K], F32, kind="ExternalInput")
    b_m0 = ncB.dram_tensor("m0", [P, EPP], F32, kind="ExternalInput")
    b_s = ncB.dram_tensor("s", [P, EPP * K], F32, kind="ExternalOutput")
    with TileContext(ncB) as tc:
        with tc.tile_pool(name="big", bufs=1) as big, tc.tile_pool(name="chp", bufs=4) as chp:
            S = big.tile([P, EPP * K], F32, tag="S")
            M0 = big.tile([P, EPP], F32, tag="M0")
            ncB.sync.dma_start(M0[:], b_m0[:])
            S3 = S[:].rearrange("p (e k) -> p e k", k=K)
            for c in range(NCH):
                a, b = c * CH, (c + 1) * CH
                mt = chp.tile([P, CH * K], F32, tag="mt")
                ncB.sync.dma_start(mt[:], b_min[:, a * K:b * K])
                ncB.scalar.activation(mt[:], mt[:], Ln)
                mt3 = mt[:].rearrange("p (e k) -> p e k", k=K)
                for kk in range(K):
                    init = 0.0 if c == 0 else S3[:, a - 1:a, kk]
                    ncB.vector.tensor_tensor_scan(
                        S3[:, a:b, kk], M0[:, a:b], mt3[:, :, kk], init, MULT, ADD)
            ncB.sync.dma_start(b_s[:], S[:])
    ncB.compile()

    _compiled[key] = (ncA, ncB)
    return _compiled[key]


_trace_ok = True


def _run_spmd(nc, in_maps):
    global _trace_ok
    from concourse.bass_utils import run_bass_kernel_spmd
    if _trace_ok:
        try:
            return run_bass_kernel_spmd(nc, in_maps,
                                        core_ids=list(range(N_CORES)), trace=True)
        except ModuleNotFoundError:
            _trace_ok = False
    return run_bass_kernel_spmd(nc, in_maps,
                                core_ids=list(range(N_CORES)), trace=False)


# --------------------------------------------------------------------------
# numpy fallback (mirrors reference exactly)
# --------------------------------------------------------------------------
def _numpy_reference(prior, W, src, dst, rev, iterations):
    n, k = prior.shape
    E = src.shape[0]
    psi = np.exp(np.clip(W, -10.0, 10.0))
    msgs = np.full((E, k), 1.0 / k, np.float32)
    for _ in range(int(iterations)):
        logm = np.log(msgs)
        logP = np.zeros((n, k), np.float32)
        np.add.at(logP, dst, logm)
        b = np.maximum(prior[src] * np.exp(logP[src] - logm[rev]), EPS)
        m = np.maximum(b @ psi, EPS)
        msgs = m / np.maximum(m.sum(-1, keepdims=True), EPS)
    logP = np.zeros((n, k), np.float32)
    np.add.at(logP, dst, np.log(msgs))
    b = np.maximum(prior * np.exp(logP), EPS)
    return (b / np.maximum(b.sum(-1, keepdims=True), EPS)).astype(np.float32)


# --------------------------------------------------------------------------
# entry point
# --------------------------------------------------------------------------
last_exec_time_ns = 0


def kernel(prior, W, src, dst, rev, iterations):
    global last_exec_time_ns
    prior = np.asarray(prior, np.float32)
    W = np.asarray(W, np.float32)
    src = np.asarray(src, np.int64)
    dst = np.asarray(dst, np.int64)
    rev = np.asarray(rev, np.int64)
    iters = int(np.asarray(iterations))
    n, k = prior.shape
    E = src.shape[0]

    psi = np.exp(np.clip(W, -10.0, 10.0)).astype(np.float64)
    alpha = float(np.diag(psi).mean())
    off = psi[~np.eye(k, dtype=bool)]
    beta = float(off.mean())
    psi_ok = (np.allclose(np.diag(psi), alpha, rtol=1e-6) and
              np.allclose(off, beta, rtol=1e-6) and alpha + 6 * beta >= 1.0)
    rev_ok = bool(np.all(rev[rev] == np.arange(E)) and np.all(dst[rev] == src)
                  and np.all(src[rev] == dst))
    if k != K or not psi_ok or not rev_ok:
        return _numpy_reference(prior, W, src, dst, rev, iters)

    try:
        return _device_path(prior, src, dst, rev, iters, alpha, beta, n)
    except Exception:
        import traceback
        traceback.print_exc()
        return _numpy_reference(prior, W, src, dst, rev, iters)


def _device_path(prior, src, dst, rev, iters, alpha, beta, n):
    global last_exec_time_ns
    lay = _build_layout(prior, src, dst, rev)
    ncA, ncB = _get_programs(alpha, beta)
    S_total = lay["S_total"]

    def percore(x, width):
        return x.reshape(N_CORES, P, width)

    m0c = percore(lay["m0"], EPP)
    nec = percore(lay["ne"], EPP)
    emc = percore(lay["em"], EPP)
    lpc = lay["lp"].reshape(N_CORES, P, EPP * K)

    M = np.full((S_total, K), 1.0 / K, np.float32)
    total_ns = 0

    for _ in range(iters):
        Mc = M.reshape(N_CORES, P, EPP * K)
        in_maps = [{"min": Mc[i], "lp": lpc[i], "m0": m0c[i],
                    "ne": nec[i], "em": emc[i]} for i in range(N_CORES)]
        res = _run_spmd(ncA, in_maps)
        if res.exec_time_ns:
            total_ns += res.exec_time_ns
            print("  launch A:", res.exec_time_ns, "ns")
        Wout = np.concatenate([res.results[i]["w"].reshape(P, EPP, K)
                               for i in range(N_CORES)], axis=0).reshape(S_total, K)
        M = Wout[lay["revslot"]]

    # final pass: segment sums of log(final msgs)
    Mc = M.reshape(N_CORES, P, EPP * K)
    in_maps = [{"min": Mc[i], "m0": m0c[i]} for i in range(N_CORES)]
    res = _run_spmd(ncB, in_maps)
    if res.exec_time_ns:
        total_ns += res.exec_time_ns
        print("  launch B:", res.exec_time_ns, "ns")
    Sarr = np.concatenate([res.results[i]["s"].reshape(P, EPP, K)
                           for i in range(N_CORES)], axis=0).reshape(S_total, K)
    runend = lay["runend_of_node"]
    logP = np.zeros((n, K), np.float32)
    has = runend >= 0
    logP[has] = Sarr[runend[has]]
    b = np.maximum(prior * np.exp(logP), EPS)
    out = b / np.maximum(b.sum(-1, keepdims=True), EPS)
    last_exec_time_ns = total_ns
    return out.astype(np.float32)

